# revision 5
# baseline (speedup 1.0000x reference)
"""DTNN layer kernel for Trainium2 (8 NeuronCores).

Math: out[b,i,o] = sum_j sum_h Wfc[o,h] * hx[b,i,h] * hd[b,i,j,h]
with hx = x@Wcf.T + bcf, hd = dist@Wdf.T + bdf.
Since Wfc/Wdf are linear, the j-sum commutes:
    ds[b,i,d]  = sum_j dist[b,i,j,d]                  (memory-bound reduction)
    out[b,i,:] = ((x@Wcf.T + bcf) * (ds@Wdf.T + N*bdf)) @ Wfc.T
So the kernel streams `distance` once (134MB) and does a few 128x128 matmuls.

Sharding: flatten (B,N) -> 1024 i-rows, 128 rows per core; no cross-core comms.

v2 structure (from NTFF trace analysis of v1):
- The HBM stream itself sustains ~400GB/s/core (all 8 cores together saturate
  the chip's ~3.2TB/s) and finishes at ~49.5us; the v1 critical path was the
  DVE fold (serial ~39us, ending ~65us) plus ~8us teardown.
- v2 splits the j-fold between DVE (~119 G elem/s measured) and GpSimd
  (~65 G elem/s per cost model, Add eff 0.42) with a per-tile ~2:1 column
  split, so each tile is folded right after it lands and the fold drains
  ~2us after the last byte arrives.
- Tile ladder shrinks toward the end so the final fold is tiny; the two
  partial accumulators are transposed straight into one PSUM bank with two
  accumulating PE transposes (no merge add).
- The (hx * N*bdf) @ WfcT bias term is preloaded into the output PSUM during
  the stream; the tail is transpose x2 -> copy -> Wdf matmul -> mul ->
  accumulate-matmul -> store.
"""

import numpy as np

import concourse.bass as bass
import concourse.bacc as bacc
import concourse.mybir as mybir
from concourse.tile import TileContext
from concourse.bass_utils import run_bass_kernel_spmd

B, N, D, H = 4, 256, 128, 128
NCORES = 8
ROWS = B * N // NCORES  # 128 i-rows per core
FP = mybir.dt.float32

# j-tile ladder and per-tile DVE share (rest goes to GpSimd). Chosen by an
# offline greedy schedule model (DVE 0.138us/slab + 0.15us/instr, GpSimd
# 0.254us/slab + 0.18us/instr, arrivals at ~400GB/s) to drain ~2us after the
# stream ends.
SIZES = [40, 40, 40, 24, 16, 16, 16, 16, 16, 16, 12, 4]
VSHARE = [26, 26, 26, 16, 10, 10, 10, 10, 10, 10, 8, 2]
assert sum(SIZES) == N and len(SIZES) == len(VSHARE)

# packed constant columns: [xT | wcfT | wdfT | wfcT | eye | rows...]
C_XT = 0
C_WCF = 128
C_WDF = 256
C_WFC = 384
C_EYE = 512
C_BCFR = 640   # partition 0: bcf row (1, H)
C_ONES = 896   # partition 0: ones row (1, ROWS)
C_BDFC = 1024  # bdf as a per-partition column (H, 1)
C_TOT = 1025


def build_nc():
    nc = bacc.Bacc("TRN2", target_bir_lowering=False)
    dist = nc.declare_dram_parameter("dist", [ROWS, N * D], FP, isOutput=False)
    cst = nc.declare_dram_parameter("cst", [128, C_TOT], FP, isOutput=False)
    out = nc.declare_dram_parameter("out", [ROWS, D], FP, isOutput=True)

    with TileContext(nc) as tc:
        with (
            tc.tile_pool(name="const", bufs=1) as cpool,
            tc.tile_pool(name="dist", bufs=1) as dpool,
            tc.tile_pool(name="work", bufs=1) as wpool,
            tc.tile_pool(name="psum", bufs=1, space="PSUM") as ppool,
        ):
            # Issue the dist stream first so the big DMAs start ASAP; the
            # constants ride on the scalar engine's ring concurrently.
            dtiles = []
            off = 0
            for k, jn in enumerate(SIZES):
                t = dpool.tile([ROWS, jn * D], FP, tag=f"dist{k}")
                nc.sync.dma_start(out=t[:], in_=dist[:, off * D:(off + jn) * D])
                dtiles.append(t)
                off += jn

            cst_t = cpool.tile([128, C_TOT], FP)
            nc.scalar.dma_start(out=cst_t[:], in_=cst[:])
            xT_t = cst_t[:, C_XT:C_XT + ROWS]
            wcf_t = cst_t[:, C_WCF:C_WCF + H]
            wdf_t = cst_t[:, C_WDF:C_WDF + H]
            wfc_t = cst_t[:, C_WFC:C_WFC + D]
            ident = cst_t[:, C_EYE:C_EYE + ROWS]
            bcf_row = cst_t[0:1, C_BCFR:C_BCFR + H]
            ones_row = cst_t[0:1, C_ONES:C_ONES + ROWS]

            # Streaming j-reduction, split per tile between DVE (slabs
            # [0:a)) and GpSimd (slabs [a:jn)). Each engine halving-folds
            # its share in place and accumulates into its own slab of
            # tile 0 (slab 0 for DVE, slab a0 for GpSimd).
            a0 = VSHARE[0]
            acc_v = dtiles[0][:, 0:D]
            acc_g = dtiles[0][:, a0 * D:(a0 + 1) * D]

            hxT = wpool.tile([H, ROWS], FP)
            bdfN = wpool.tile([H, 1], FP)
            s0T = wpool.tile([H, ROWS], FP)
            hx_ps = ppool.tile([H, ROWS], FP)
            out_ps = ppool.tile([ROWS, D], FP)

            def fold(eng, t, s, n, acc, k):
                # halving-fold slabs [s, s+n) of tile t in place; result in
                # slab s; accumulate into acc for k > 0.
                while n > 1:
                    h = n // 2
                    eng.tensor_add(
                        t[:, s * D:(s + h) * D],
                        t[:, s * D:(s + h) * D],
                        t[:, (s + n - h) * D:(s + n) * D],
                    )
                    n -= h
                if k > 0:
                    eng.tensor_add(acc, acc, t[:, s * D:(s + 1) * D])

            for k, jn in enumerate(SIZES):
                a = VSHARE[k]
                fold(nc.vector, dtiles[k], 0, a, acc_v, k)
                fold(nc.gpsimd, dtiles[k], a, jn - a, acc_g, k)
                if k == 2:
                    # hx^T = (Wcf^T)^T @ x^T + bcf x ones -> (H, ROWS).
                    # Issued mid-ladder: by the time the engines reach these
                    # ops the cst DMA (lands ~21us) has arrived, so they
                    # never stall the fold pipeline.
                    nc.tensor.matmul(hx_ps[:], wcf_t, xT_t,
                                     start=True, stop=False)
                    nc.tensor.matmul(hx_ps[:], bcf_row, ones_row,
                                     start=False, stop=True)
                    nc.vector.tensor_copy(hxT[:], hx_ps[:])
                    # Bias term s0^T = hx^T * N*bdf, kept in SBUF and added
                    # to s^T in the tail (avoids a split PSUM accumulation
                    # group, which the scheduler may reorder).
                    nc.gpsimd.tensor_scalar_mul(
                        bdfN[:], cst_t[:, C_BDFC:C_BDFC + 1], float(N))
                    nc.gpsimd.tensor_scalar_mul(s0T[:], hxT[:], bdfN[:])

            # merge the two partial accumulators, then ds -> ds^T via PE
            nc.vector.tensor_add(acc_v, acc_v, acc_g)
            dsT_ps = ppool.tile([D, ROWS], FP)
            nc.tensor.transpose(dsT_ps[:], acc_v, ident)
            dsT = wpool.tile([D, ROWS], FP)
            nc.vector.tensor_copy(dsT[:], dsT_ps[:])

            # hd^T (bias-free) = (Wdf^T)^T @ ds^T -> (H, ROWS)
            hd_ps = ppool.tile([H, ROWS], FP)
            nc.tensor.matmul(hd_ps[:], wdf_t, dsT[:], start=True, stop=True)

            # s^T = hx^T * hd^T + s0^T (one PSUM operand max per DVE op)
            sT = wpool.tile([H, ROWS], FP)
            nc.vector.tensor_mul(sT[:], hd_ps[:], hxT[:])
            nc.vector.tensor_add(sT[:], sT[:], s0T[:])

            # out = sT^T @ Wfc^T
            nc.tensor.matmul(out_ps[:], sT[:], wfc_t, start=True, stop=True)
            out_sb = wpool.tile([ROWS, D], FP)
            nc.vector.tensor_copy(out_sb[:], out_ps[:])
            nc.sync.dma_start(out=out[:], in_=out_sb[:])
    nc.compile()
    return nc


_NC_CACHE = None


def _get_nc():
    global _NC_CACHE
    if _NC_CACHE is None:
        _NC_CACHE = build_nc()
    return _NC_CACHE


def _make_in_maps(x, distance, Wcf_w, Wcf_b, Wdf_w, Wdf_b, Wfc_w):
    x = np.ascontiguousarray(np.asarray(x, np.float32))
    distance = np.ascontiguousarray(np.asarray(distance, np.float32))
    x_flat = x.reshape(B * N, D)
    dist_flat = distance.reshape(B * N, N * D)
    wcfT = np.asarray(Wcf_w, np.float32).T
    wdfT = np.asarray(Wdf_w, np.float32).T
    wfcT = np.asarray(Wfc_w, np.float32).T
    bcf = np.asarray(Wcf_b, np.float32)
    bdf = np.asarray(Wdf_b, np.float32)
    in_maps = []
    for c in range(NCORES):
        sl = slice(c * ROWS, (c + 1) * ROWS)
        cstblk = np.zeros((128, C_TOT), np.float32)
        cstblk[:, C_XT:C_XT + ROWS] = x_flat[sl].T
        cstblk[:, C_WCF:C_WCF + H] = wcfT
        cstblk[:, C_WDF:C_WDF + H] = wdfT
        cstblk[:, C_WFC:C_WFC + D] = wfcT
        cstblk[:, C_EYE:C_EYE + ROWS] = np.eye(ROWS, dtype=np.float32)
        cstblk[0, C_BCFR:C_BCFR + H] = bcf
        cstblk[0, C_ONES:C_ONES + ROWS] = 1.0
        cstblk[:, C_BDFC] = bdf
        in_maps.append({
            "dist": np.ascontiguousarray(dist_flat[sl]),
            "cst": cstblk,
        })
    return in_maps


def kernel(x, distance, Wcf_w, Wcf_b, Wdf_w, Wdf_b, Wfc_w):
    in_maps = _make_in_maps(x, distance, Wcf_w, Wcf_b, Wdf_w, Wdf_b, Wfc_w)
    nc = _get_nc()
    res = run_bass_kernel_spmd(nc, in_maps, list(range(NCORES))).results
    out = np.concatenate([res[c]["out"] for c in range(NCORES)], axis=0)
    return out.reshape(B, N, D)


# revision 6
# speedup vs baseline: 1.7426x; 1.7426x over previous
"""DTNN layer kernel for Trainium2 (8 NeuronCores).

Math: out[b,i,o] = sum_j sum_h Wfc[o,h] * hx[b,i,h] * hd[b,i,j,h]
with hx = x@Wcf.T + bcf, hd = dist@Wdf.T + bdf.
Since Wfc/Wdf are linear, the j-sum commutes:
    ds[b,i,d]  = sum_j dist[b,i,j,d]                  (memory-bound reduction)
    out[b,i,:] = ((x@Wcf.T + bcf) * (ds@Wdf.T + N*bdf)) @ Wfc.T
So the kernel streams `distance` once and does a few 128x128 matmuls.

Sharding: flatten (B,N) -> 1024 i-rows, 128 rows per core; no cross-core comms.

v3 (from NTFF trace analysis of v1/v2):
- `distance` is streamed as bf16 (host-side cast). The result tolerance is
  2e-2 and the j-sum of 256 ~N(0,1) values loses ~0.3% relative accuracy to
  bf16 rounding - far inside tolerance - while halving both the HBM bytes
  (the stream is the roofline) and doubling the DVE fold rate (16-bit DVE
  ops run 2 elem/lane/cycle).
- The j-fold is DVE-only: v2 showed GpSimd tensor ops run ~4x slower than
  the cost model and degrade both the stream and DVE throughput while
  active (SBUF/arbitration contention), so the Pool engine stays idle.
- Per-tile halving fold in bf16, accumulated into an fp32 accumulator; the
  tail (transpose -> Wdf matmul -> mul -> out matmul) stays fp32.
- cst is split into an early DMA (xT, Wcf, biases - needed for hx during
  the stream) and a late DMA (Wdf, Wfc, identity - needed only in the
  tail), both on the scalar engine's ring.
"""

import numpy as np
import ml_dtypes

import concourse.bass as bass
import concourse.bacc as bacc
import concourse.mybir as mybir
from concourse.tile import TileContext
from concourse.bass_utils import run_bass_kernel_spmd

B, N, D, H = 4, 256, 128, 128
NCORES = 8
ROWS = B * N // NCORES  # 128 i-rows per core
FP = mybir.dt.float32
BF = mybir.dt.bfloat16

# j-tile ladder (bf16 arrivals ~0.08us/j at ~400GB/s; DVE bf16 fold
# ~0.069us/slab + ~0.15us/instr). Fold is slightly work-bound, so a few
# big tiles minimize instruction overhead.
SIZES = [32, 56, 56, 56, 56]
assert sum(SIZES) == N

# packed constant columns (fp32):
# early DMA: [xT | wcfT | bcf_row | ones_row | bdf_col]
# late DMA:  [wdfT | wfcT | eye]
C_XT = 0
C_WCF = 128
C_BCFR = 256   # partition 0: bcf row (1, H)
C_ONES = 384   # partition 0: ones row (1, ROWS)
C_BDFC = 512   # bdf as a per-partition column (H, 1)
C_EARLY = 513
C_WDF = 513
C_WFC = 641
C_EYE = 769
C_TOT = 897


def build_nc():
    nc = bacc.Bacc("TRN2", target_bir_lowering=False)
    dist = nc.declare_dram_parameter("dist", [ROWS, N * D], BF, isOutput=False)
    cst = nc.declare_dram_parameter("cst", [128, C_TOT], FP, isOutput=False)
    out = nc.declare_dram_parameter("out", [ROWS, D], FP, isOutput=True)

    with TileContext(nc) as tc:
        with (
            tc.tile_pool(name="const", bufs=1) as cpool,
            tc.tile_pool(name="dist", bufs=1) as dpool,
            tc.tile_pool(name="work", bufs=1) as wpool,
            tc.tile_pool(name="psum", bufs=1, space="PSUM") as ppool,
        ):
            # Issue the dist stream first so the big DMAs start ASAP; the
            # constants ride on the scalar engine's ring concurrently.
            dtiles = []
            off = 0
            for k, jn in enumerate(SIZES):
                t = dpool.tile([ROWS, jn * D], BF, tag=f"dist{k}")
                nc.sync.dma_start(out=t[:], in_=dist[:, off * D:(off + jn) * D])
                dtiles.append(t)
                off += jn

            cst_t = cpool.tile([128, C_TOT], FP)
            nc.scalar.dma_start(out=cst_t[:, 0:C_EARLY], in_=cst[:, 0:C_EARLY])
            nc.scalar.dma_start(out=cst_t[:, C_EARLY:], in_=cst[:, C_EARLY:])
            xT_t = cst_t[:, C_XT:C_XT + ROWS]
            wcf_t = cst_t[:, C_WCF:C_WCF + H]
            wdf_t = cst_t[:, C_WDF:C_WDF + H]
            wfc_t = cst_t[:, C_WFC:C_WFC + D]
            ident = cst_t[:, C_EYE:C_EYE + ROWS]
            bcf_row = cst_t[0:1, C_BCFR:C_BCFR + H]
            ones_row = cst_t[0:1, C_ONES:C_ONES + ROWS]

            hxT = wpool.tile([H, ROWS], FP)
            bdfN = wpool.tile([H, 1], FP)
            s0T = wpool.tile([H, ROWS], FP)
            hx_ps = ppool.tile([H, ROWS], FP)
            acc = wpool.tile([ROWS, D], FP)

            # Streaming j-reduction on DVE: halving-fold each tile in place
            # (bf16), then accumulate the 128-wide result into fp32 acc.
            for k, jn in enumerate(SIZES):
                t = dtiles[k]
                n = jn
                while n > 1:
                    h = n // 2
                    nc.vector.tensor_add(
                        t[:, 0:h * D],
                        t[:, 0:h * D],
                        t[:, (n - h) * D:n * D],
                    )
                    n -= h
                if k == 0:
                    nc.vector.tensor_copy(acc[:], t[:, 0:D])
                else:
                    nc.vector.tensor_add(acc[:], acc[:], t[:, 0:D])
                if k == 1:
                    # hx^T = (Wcf^T)^T @ x^T + bcf x ones -> (H, ROWS).
                    # Issued mid-ladder: the early-cst DMA (~14us) lands
                    # before DVE/PE reach these ops.
                    nc.tensor.matmul(hx_ps[:], wcf_t, xT_t,
                                     start=True, stop=False)
                    nc.tensor.matmul(hx_ps[:], bcf_row, ones_row,
                                     start=False, stop=True)
                    nc.vector.tensor_copy(hxT[:], hx_ps[:])
                    # Bias term s0^T = hx^T * N*bdf (added to s^T in the tail)
                    nc.vector.tensor_scalar_mul(
                        bdfN[:], cst_t[:, C_BDFC:C_BDFC + 1], float(N))
                    nc.vector.tensor_scalar_mul(s0T[:], hxT[:], bdfN[:])

            # ds -> ds^T via PE
            dsT_ps = ppool.tile([D, ROWS], FP)
            nc.tensor.transpose(dsT_ps[:], acc[:], ident)
            dsT = wpool.tile([D, ROWS], FP)
            nc.vector.tensor_copy(dsT[:], dsT_ps[:])

            # hd^T (bias-free) = (Wdf^T)^T @ ds^T -> (H, ROWS)
            hd_ps = ppool.tile([H, ROWS], FP)
            nc.tensor.matmul(hd_ps[:], wdf_t, dsT[:], start=True, stop=True)

            # s^T = hx^T * hd^T + s0^T (one PSUM operand max per DVE op)
            sT = wpool.tile([H, ROWS], FP)
            nc.vector.tensor_mul(sT[:], hd_ps[:], hxT[:])
            nc.vector.tensor_add(sT[:], sT[:], s0T[:])

            # out = sT^T @ Wfc^T
            out_ps = ppool.tile([ROWS, D], FP)
            nc.tensor.matmul(out_ps[:], sT[:], wfc_t, start=True, stop=True)
            out_sb = wpool.tile([ROWS, D], FP)
            nc.vector.tensor_copy(out_sb[:], out_ps[:])
            nc.sync.dma_start(out=out[:], in_=out_sb[:])
    nc.compile()
    return nc


_NC_CACHE = None


def _get_nc():
    global _NC_CACHE
    if _NC_CACHE is None:
        _NC_CACHE = build_nc()
    return _NC_CACHE


def _make_in_maps(x, distance, Wcf_w, Wcf_b, Wdf_w, Wdf_b, Wfc_w):
    x = np.ascontiguousarray(np.asarray(x, np.float32))
    distance = np.asarray(distance, np.float32)
    x_flat = x.reshape(B * N, D)
    dist_flat = np.ascontiguousarray(
        distance.reshape(B * N, N * D).astype(ml_dtypes.bfloat16))
    wcfT = np.asarray(Wcf_w, np.float32).T
    wdfT = np.asarray(Wdf_w, np.float32).T
    wfcT = np.asarray(Wfc_w, np.float32).T
    bcf = np.asarray(Wcf_b, np.float32)
    bdf = np.asarray(Wdf_b, np.float32)
    in_maps = []
    for c in range(NCORES):
        sl = slice(c * ROWS, (c + 1) * ROWS)
        cstblk = np.zeros((128, C_TOT), np.float32)
        cstblk[:, C_XT:C_XT + ROWS] = x_flat[sl].T
        cstblk[:, C_WCF:C_WCF + H] = wcfT
        cstblk[0, C_BCFR:C_BCFR + H] = bcf
        cstblk[0, C_ONES:C_ONES + ROWS] = 1.0
        cstblk[:, C_BDFC] = bdf
        cstblk[:, C_WDF:C_WDF + H] = wdfT
        cstblk[:, C_WFC:C_WFC + D] = wfcT
        cstblk[:, C_EYE:C_EYE + ROWS] = np.eye(ROWS, dtype=np.float32)
        in_maps.append({
            "dist": np.ascontiguousarray(dist_flat[sl]),
            "cst": cstblk,
        })
    return in_maps


def kernel(x, distance, Wcf_w, Wcf_b, Wdf_w, Wdf_b, Wfc_w):
    in_maps = _make_in_maps(x, distance, Wcf_w, Wcf_b, Wdf_w, Wdf_b, Wfc_w)
    nc = _get_nc()
    res = run_bass_kernel_spmd(nc, in_maps, list(range(NCORES))).results
    out = np.concatenate([res[c]["out"] for c in range(NCORES)], axis=0)
    return out.reshape(B, N, D)


# revision 7
# speedup vs baseline: 1.9407x; 1.1137x over previous
"""DTNN layer kernel for Trainium2 (8 NeuronCores).

Math: out[b,i,o] = sum_j sum_h Wfc[o,h] * hx[b,i,h] * hd[b,i,j,h]
with hx = x@Wcf.T + bcf, hd = dist@Wdf.T + bdf.
Since Wfc/Wdf are linear, the j-sum commutes:
    ds[b,i,d]  = sum_j dist[b,i,j,d]                  (memory-bound reduction)
    out[b,i,:] = ((x@Wcf.T + bcf) * (ds@Wdf.T + N*bdf)) @ Wfc.T
So the kernel streams `distance` once and does a few 128x128 matmuls.

Sharding: flatten (B,N) -> 1024 i-rows, 128 rows per core; no cross-core comms.

v4 (from NTFF traces of v1-v3):
- `distance` streams as bf16 (host cast): halves HBM bytes (the roofline)
  and doubles the DVE fold rate (16-bit = 2 elem/lane/cycle, 72ns/slab
  measured). Fold error ~0.3% rel, far inside the 2e-2 tolerance.
- DVE does ONLY the j-fold ladder plus two tail ops. All copies/scales run
  on the otherwise-idle Scalar (ACT) engine so the cst DMA can never stall
  the fold stream (v3 lost ~5us to the scheduler hoisting cst-gated ops to
  the front of the DVE queue). GpSimd stays idle (v2: its software tensor
  ops are ~4x slower than modeled and degrade DMA+DVE while active).
- Tail matmuls run in bf16 (weights cast on-device during the stream);
  the bias term (hx*N*bdf)@WfcT is precomputed into SBUF during the
  stream, and the final DVE add fuses PSUM-read + bias-add + SBUF-write.
"""

import numpy as np
import ml_dtypes

import concourse.bass as bass
import concourse.bacc as bacc
import concourse.mybir as mybir
from concourse.tile import TileContext
from concourse.bass_utils import run_bass_kernel_spmd

B, N, D, H = 4, 256, 128, 128
NCORES = 8
ROWS = B * N // NCORES  # 128 i-rows per core
FP = mybir.dt.float32
BF = mybir.dt.bfloat16

# j-tile ladder: bf16 arrivals ~0.085us/j at ~390GB/s; DVE bf16 fold
# 72ns/slab + ~155ns/instr. The fold is rate-matched to the stream, so a
# few big tiles minimize instruction overhead; schedule-model spread
# across ladder shapes is <0.3us.
SIZES = [64, 64, 64, 64]
assert sum(SIZES) == N

# packed constant columns (fp32):
# early DMA: [xT | wcfT | bcf_row | ones_row | bdf_col]
# late DMA:  [wdfT | wfcT | eye]
C_XT = 0
C_WCF = 128
C_BCFR = 256   # partition 0: bcf row (1, H)
C_ONES = 384   # partition 0: ones row (1, ROWS)
C_BDFC = 512   # bdf as a per-partition column (H, 1)
C_EARLY = 513
C_WDF = 513
C_WFC = 641
C_EYE = 769
C_TOT = 897
COPY = mybir.ActivationFunctionType.Copy


def build_nc():
    nc = bacc.Bacc("TRN2", target_bir_lowering=False)
    dist = nc.declare_dram_parameter("dist", [ROWS, N * D], BF, isOutput=False)
    cst = nc.declare_dram_parameter("cst", [128, C_TOT], FP, isOutput=False)
    out = nc.declare_dram_parameter("out", [ROWS, D], FP, isOutput=True)

    with TileContext(nc) as tc:
        with (
            tc.tile_pool(name="const", bufs=1) as cpool,
            tc.tile_pool(name="dist", bufs=1) as dpool,
            tc.tile_pool(name="work", bufs=1) as wpool,
            tc.tile_pool(name="psum", bufs=1, space="PSUM") as ppool,
        ):
            # Issue the dist stream first so the big DMAs start ASAP; the
            # constants ride on the scalar engine's ring concurrently.
            dtiles = []
            off = 0
            for k, jn in enumerate(SIZES):
                t = dpool.tile([ROWS, jn * D], BF, tag=f"dist{k}")
                nc.sync.dma_start(out=t[:], in_=dist[:, off * D:(off + jn) * D])
                dtiles.append(t)
                off += jn

            cst_t = cpool.tile([128, C_TOT], FP)
            nc.scalar.dma_start(out=cst_t[:, 0:C_EARLY], in_=cst[:, 0:C_EARLY])
            nc.scalar.dma_start(out=cst_t[:, C_EARLY:], in_=cst[:, C_EARLY:])
            xT_t = cst_t[:, C_XT:C_XT + ROWS]
            wcf_t = cst_t[:, C_WCF:C_WCF + H]
            wdf_t = cst_t[:, C_WDF:C_WDF + H]
            wfc_t = cst_t[:, C_WFC:C_WFC + D]
            ident = cst_t[:, C_EYE:C_EYE + ROWS]
            bcf_row = cst_t[0:1, C_BCFR:C_BCFR + H]
            ones_row = cst_t[0:1, C_ONES:C_ONES + ROWS]

            acc = wpool.tile([ROWS, D], FP)

            # Streaming j-reduction on DVE only: halving-fold each tile in
            # place (bf16); fold results accumulate into fp32 acc (the
            # first accumulate fuses the bf16->fp32 cast).
            for k, jn in enumerate(SIZES):
                t = dtiles[k]
                n = jn
                while n > 1:
                    h = n // 2
                    nc.vector.tensor_add(
                        t[:, 0:h * D],
                        t[:, 0:h * D],
                        t[:, (n - h) * D:n * D],
                    )
                    n -= h
                if k == 1:
                    nc.vector.tensor_add(acc[:], dtiles[0][:, 0:D], t[:, 0:D])
                elif k > 1:
                    nc.vector.tensor_add(acc[:], acc[:], t[:, 0:D])

            # hx^T = (Wcf^T)^T @ x^T + bcf x ones -> (H, ROWS), on PE +
            # Scalar only (never blocks the DVE fold queue).
            hx_ps = ppool.tile([H, ROWS], FP)
            nc.tensor.matmul(hx_ps[:], wcf_t, xT_t, start=True, stop=False)
            nc.tensor.matmul(hx_ps[:], bcf_row, ones_row,
                             start=False, stop=True)
            hxT = wpool.tile([H, ROWS], FP)
            nc.scalar.activation(hxT[:], hx_ps[:], COPY)

            # Bias term (hx * N*bdf) @ Wfc^T precomputed into SBUF during
            # the stream; the final DVE add applies it.
            bdfN = wpool.tile([H, 1], FP)
            nc.scalar.activation(bdfN[:], cst_t[:, C_BDFC:C_BDFC + 1], COPY,
                                 scale=float(N))
            s0T = wpool.tile([H, ROWS], FP)
            nc.scalar.activation(s0T[:], hxT[:], COPY, scale=bdfN[:, 0:1])
            out2_ps = ppool.tile([ROWS, D], FP)
            nc.tensor.matmul(out2_ps[:], s0T[:], wfc_t, start=True, stop=True)
            out2 = wpool.tile([ROWS, D], FP)
            nc.scalar.activation(out2[:], out2_ps[:], COPY)

            # bf16 copies of the tail weights (cast on-device, off-path)
            wdf16 = wpool.tile([H, H], BF)
            nc.scalar.activation(wdf16[:], wdf_t, COPY)
            wfc16 = wpool.tile([H, D], BF)
            nc.scalar.activation(wfc16[:], wfc_t, COPY)

            # ds -> ds^T via PE (fp32), then bf16 copy on Scalar
            dsT_ps = ppool.tile([D, ROWS], FP)
            nc.tensor.transpose(dsT_ps[:], acc[:], ident)
            dsT = wpool.tile([D, ROWS], BF)
            nc.scalar.activation(dsT[:], dsT_ps[:], COPY)

            # hd^T (bias-free) = (Wdf^T)^T @ ds^T -> (H, ROWS), bf16
            hd_ps = ppool.tile([H, ROWS], FP)
            nc.tensor.matmul(hd_ps[:], wdf16[:], dsT[:], start=True, stop=True)

            # s^T = hx^T * hd^T (bf16 out; one PSUM operand max per DVE op)
            sT = wpool.tile([H, ROWS], BF)
            nc.vector.tensor_mul(sT[:], hd_ps[:], hxT[:])

            # out = sT^T @ Wfc^T + out2 (the DVE add fuses PSUM read,
            # bias add and SBUF write)
            out_ps = ppool.tile([ROWS, D], FP)
            nc.tensor.matmul(out_ps[:], sT[:], wfc16[:], start=True, stop=True)
            out_sb = wpool.tile([ROWS, D], FP)
            nc.vector.tensor_add(out_sb[:], out_ps[:], out2[:])
            nc.sync.dma_start(out=out[:], in_=out_sb[:])
    nc.compile()
    return nc


_NC_CACHE = None


def _get_nc():
    global _NC_CACHE
    if _NC_CACHE is None:
        _NC_CACHE = build_nc()
    return _NC_CACHE


def _make_in_maps(x, distance, Wcf_w, Wcf_b, Wdf_w, Wdf_b, Wfc_w):
    x = np.ascontiguousarray(np.asarray(x, np.float32))
    distance = np.asarray(distance, np.float32)
    x_flat = x.reshape(B * N, D)
    dist_flat = np.ascontiguousarray(
        distance.reshape(B * N, N * D).astype(ml_dtypes.bfloat16))
    wcfT = np.asarray(Wcf_w, np.float32).T
    wdfT = np.asarray(Wdf_w, np.float32).T
    wfcT = np.asarray(Wfc_w, np.float32).T
    bcf = np.asarray(Wcf_b, np.float32)
    bdf = np.asarray(Wdf_b, np.float32)
    in_maps = []
    for c in range(NCORES):
        sl = slice(c * ROWS, (c + 1) * ROWS)
        cstblk = np.zeros((128, C_TOT), np.float32)
        cstblk[:, C_XT:C_XT + ROWS] = x_flat[sl].T
        cstblk[:, C_WCF:C_WCF + H] = wcfT
        cstblk[0, C_BCFR:C_BCFR + H] = bcf
        cstblk[0, C_ONES:C_ONES + ROWS] = 1.0
        cstblk[:, C_BDFC] = bdf
        cstblk[:, C_WDF:C_WDF + H] = wdfT
        cstblk[:, C_WFC:C_WFC + D] = wfcT
        cstblk[:, C_EYE:C_EYE + ROWS] = np.eye(ROWS, dtype=np.float32)
        in_maps.append({
            "dist": np.ascontiguousarray(dist_flat[sl]),
            "cst": cstblk,
        })
    return in_maps


def kernel(x, distance, Wcf_w, Wcf_b, Wdf_w, Wdf_b, Wfc_w):
    in_maps = _make_in_maps(x, distance, Wcf_w, Wcf_b, Wdf_w, Wdf_b, Wfc_w)
    nc = _get_nc()
    res = run_bass_kernel_spmd(nc, in_maps, list(range(NCORES))).results
    out = np.concatenate([res[c]["out"] for c in range(NCORES)], axis=0)
    return out.reshape(B, N, D)
